# revision 13
# baseline (speedup 1.0000x reference)
"""Causal Group-Query Attention kernel for Trainium2 (8 NeuronCores, SPMD).

Problem: x[2,2048,2048] @ Wq -> q(16 heads x 128); x @ Wkv -> k,v (4 KV heads);
causal softmax attention with GQA (4 q-heads per kv-head); y @ Wc -> out.

Sharding (2 batch x 4 head-groups = 8 cores):
  core = 4*b + g handles batch b, q-heads 4g..4g+3 (= kv head g).
  Each core gets xT (x[b] transposed, [C,T]), its Wq/Wk/Wv column shards and
  Wc row shard, and produces a partial [T,C] output; host sums the 4 partials
  per batch (the "all-reduce" of the c_proj happens on host).

Per-core device pipeline, software-pipelined over 512-wide t strips
(matmuls in f32r = fp32 HIGH mode, ~tf32 precision at near-bf16 rate):
  per strip s: projections (qT strip per head, kT strip, v strip via
  PE transpose); then per head: S^T blocks [tk=128, tq=512], exp on ScalarE
  (softmax scale fused), causal diagonal masks on DVE, denominator row via
  ones-column matmul accumulation, yT via matmul(lhsT=v_block, rhs=p_block),
  normalization (denom -> PE outer-product broadcast -> DVE reciprocal ->
  multiply); then c_proj for the strip's 4 t-tiles, DMA out.
All PSUM accumulators share one 4-slot pool; S^T pairs use a 2x2-bank pool.
"""

import sys

sys.path.insert(0, "/opt/trn_rl_repo")

import numpy as np

import concourse.bass as bass  # noqa: F401
import concourse.tile as tile
from concourse import bacc, mybir
from concourse.masks import make_identity

F32 = mybir.dt.float32
F32R = mybir.dt.float32r

T_FULL = 2048
C = 2048          # model dim (contraction for projections)
D = 128           # head dim
HPC = 4           # heads per core
P = 128
CI = C // P       # 16 contraction tiles
CG = 8            # ci-tiles per xt half-tile
SCALE = 1.0 / float(np.sqrt(D))


def build_nc(T=T_FULL):
    """Build and compile the per-core Bass module. T: multiple of 512."""
    assert T % 512 == 0
    TS = T // 512

    nc = bacc.Bacc("TRN2", target_bir_lowering=False, debug=False,
                   enable_asserts=True, num_devices=8)

    xt_d = nc.dram_tensor("xt", [C, T], F32R, kind="ExternalInput").ap()
    wq_d = nc.dram_tensor("wq", [C, HPC * D], F32R, kind="ExternalInput").ap()
    wk_d = nc.dram_tensor("wk", [C, D], F32R, kind="ExternalInput").ap()
    wv_d = nc.dram_tensor("wv", [C, D], F32R, kind="ExternalInput").ap()
    wc_d = nc.dram_tensor("wc", [HPC * D, C], F32R, kind="ExternalInput").ap()
    mask_d = nc.dram_tensor("mask", [4, P, 512], F32R, kind="ExternalInput").ap()
    ones_d = nc.dram_tensor("ones", [P, P], F32R, kind="ExternalInput").ap()
    out_d = nc.dram_tensor("out", [T, C], F32, kind="ExternalOutput").ap()

    xt_v = xt_d.rearrange("(ci p) t -> p ci t", p=P)          # [128,16,T]
    wq_v = wq_d.rearrange("(ci p) e -> p ci e", p=P)          # [128,16,512]
    wk_v = wk_d.rearrange("(ci p) d -> p ci d", p=P)          # [128,16,128]
    wv_v = wv_d.rearrange("(ci p) d -> p ci d", p=P)          # [128,16,128]
    wc_v = wc_d.rearrange("(hh p) o -> p hh o", p=P)          # [128,4,C]
    mask_v = mask_d.rearrange("b p c -> p b c")               # [128,4,512]
    out_v = out_d.rearrange("(tt p) (os o) -> p tt os o", p=P, o=512)

    with tile.TileContext(nc) as tc:
        with (
            tc.tile_pool(name="consts", bufs=1) as consts,
            tc.tile_pool(name="weights", bufs=1) as weights,
            tc.tile_pool(name="persist", bufs=1) as persist,
            tc.tile_pool(name="xtp", bufs=2) as xtp,
            tc.tile_pool(name="qtp", bufs=2) as qtp,
            tc.tile_pool(name="ytp", bufs=2) as ytp,
            tc.tile_pool(name="vts", bufs=1) as vts,
            tc.tile_pool(name="pp", bufs=3) as pp,
            tc.tile_pool(name="np_", bufs=2) as np_,
            tc.tile_pool(name="op", bufs=2) as op,
            tc.tile_pool(name="acc", bufs=4, space="PSUM") as acc,
            tc.tile_pool(name="sp", bufs=2, space="PSUM") as sp,
        ):
            # --- weights / consts; first strip's x chunks get priority ---
            # (emission order = DMA issue order: x(s0)+wq chunks first so the
            # first projection matmuls start ~6us in, not after all inputs)
            xt_sb0 = [xtp.tile([P, CG, 512], F32R, tag="xt", name=f"xt0_{i}")
                      for i in range(CI // CG)]
            wq_sb = weights.tile([P, CI, HPC * D], F32R, tag="wq")
            for q in range(4):
                qsl = slice(q * 4, (q + 1) * 4)
                nc.sync.dma_start(xt_sb0[q // 2][:, (q % 2) * 4:(q % 2) * 4 + 4, :],
                                  xt_v[:, qsl, 0:512])
                nc.sync.dma_start(wq_sb[:, qsl, :], wq_v[:, qsl, :])
            wk_sb = weights.tile([P, CI, D], F32R, tag="wk")
            nc.sync.dma_start(wk_sb[:], wk_v)
            wv_sb = weights.tile([P, CI, D], F32R, tag="wv")
            nc.sync.dma_start(wv_sb[:], wv_v)
            mask_sb = consts.tile([P, 4, 512], F32R, tag="mask")
            nc.sync.dma_start(mask_sb[:], mask_v)
            ones_sb = consts.tile([P, P], F32R, tag="ones")
            nc.sync.dma_start(ones_sb[:], ones_d)
            ones_col = ones_sb[:, 0:1]
            ones_row = ones_sb[0:1, :]
            ident = consts.tile([P, P], F32, tag="ident")
            make_identity(nc, ident[:])
            wc_sb = weights.tile([P, HPC, C], F32R, tag="wc")
            for cg in range(2):
                nc.sync.dma_start(wc_sb[:, :, cg * C // 2:(cg + 1) * C // 2],
                                  wc_v[:, :, cg * C // 2:(cg + 1) * C // 2])

            kt_sb = persist.tile([P, T], F32R, tag="kt")        # [d, t]
            v_sb = persist.tile([P, T // P, D], F32R, tag="v")  # [t, tt, d]

            for s in range(TS):
                sl = slice(s * 512, (s + 1) * 512)
                if s == 0:
                    xt_sb = xt_sb0
                else:
                    xt_sb = [xtp.tile([P, CG, 512], F32R, tag="xt",
                                      name=f"xt{s}_{i}")
                             for i in range(CI // CG)]
                    for q in range(4):
                        nc.sync.dma_start(
                            xt_sb[q // 2][:, (q % 2) * 4:(q % 2) * 4 + 4, :],
                            xt_v[:, q * 4:(q + 1) * 4, sl])

                # ---- projections for strip s ----
                ps = acc.tile([P, 512], F32, tag="acc")          # kT strip
                for ci in range(CI):
                    nc.tensor.matmul(
                        ps[:], lhsT=wk_sb[:, ci, :],
                        rhs=xt_sb[ci // CG][:, ci % CG, :],
                        start=(ci == 0), stop=(ci == CI - 1))
                nc.vector.tensor_copy(out=kt_sb[:, sl], in_=ps[:])

                qt_sb = qtp.tile([P, HPC, 512], F32R, tag="qt")  # [d, h, tq]
                for e in range(HPC):
                    ps = acc.tile([P, 512], F32, tag="acc")
                    for ci in range(CI):
                        nc.tensor.matmul(
                            ps[:], lhsT=wq_sb[:, ci, e * D:(e + 1) * D],
                            rhs=xt_sb[ci // CG][:, ci % CG, :],
                            start=(ci == 0), stop=(ci == CI - 1))
                    nc.vector.tensor_copy(out=qt_sb[:, e, :], in_=ps[:])

                ps = acc.tile([P, 512], F32, tag="acc")          # vT strip
                for ci in range(CI):
                    nc.tensor.matmul(
                        ps[:], lhsT=wv_sb[:, ci, :],
                        rhs=xt_sb[ci // CG][:, ci % CG, :],
                        start=(ci == 0), stop=(ci == CI - 1))
                vt_sb = vts.tile([P, 512], F32, tag="vt")
                nc.vector.tensor_copy(out=vt_sb[:], in_=ps[:])
                for k in range(4):    # PE transpose -> v natural [t, d]
                    tp = acc.tile([P, P], F32, tag="acc")
                    nc.tensor.transpose(tp[:], vt_sb[:, k * P:(k + 1) * P],
                                        ident[:])
                    nc.vector.tensor_copy(out=v_sb[:, s * 4 + k, :], in_=tp[:])

                # ---- attention for strip s, all heads ----
                yt_sb = ytp.tile([P, HPC, 512], F32R, tag="yt")  # [d, h, tq]
                for h in range(HPC):
                    nblk = 4 * s + 4      # causal: tk tiles j = 0..nblk-1
                    yt_ps = acc.tile([P, 512], F32, tag="acc")
                    dn_ps = acc.tile([1, 512], F32, tag="acc")
                    for jp in range(0, nblk, 2):
                        s_ps = sp.tile([P, 2, 512], F32, tag="s_ps")
                        for u in range(2):
                            j = jp + u
                            nc.tensor.matmul(
                                s_ps[:, u, :],
                                lhsT=kt_sb[:, j * P:(j + 1) * P],
                                rhs=qt_sb[:, h, :],
                                start=True, stop=True)
                        p_sb = pp.tile([P, 2, 512], F32R, tag="p_sb")
                        nc.scalar.activation(
                            p_sb[:], s_ps[:],
                            mybir.ActivationFunctionType.Exp, scale=SCALE)
                        for u in range(2):
                            b = jp + u - 4 * s
                            if b >= 0:    # diagonal block: causal mask
                                nc.vector.tensor_mul(
                                    out=p_sb[:, u, :], in0=p_sb[:, u, :],
                                    in1=mask_sb[:, b, :])
                        for u in range(2):
                            j = jp + u
                            nc.tensor.matmul(
                                yt_ps[:], lhsT=v_sb[:, j, :],
                                rhs=p_sb[:, u, :],
                                start=(j == 0), stop=(j == nblk - 1))
                            nc.tensor.matmul(
                                dn_ps[:], lhsT=ones_col,
                                rhs=p_sb[:, u, :],
                                start=(j == 0), stop=(j == nblk - 1))
                    # normalize: PE broadcast of denom row, DVE recip, mul
                    dnrow_sb = np_.tile([1, 512], F32R, tag="dnrow")
                    nc.scalar.copy(out=dnrow_sb[:], in_=dn_ps[:])
                    bc_ps = acc.tile([P, 512], F32, tag="acc")
                    nc.tensor.matmul(
                        bc_ps[:], lhsT=ones_row,
                        rhs=dnrow_sb[:], start=True, stop=True)
                    drecip = np_.tile([P, 512], F32, tag="drecip")
                    nc.vector.reciprocal_approx_fast(out=drecip[:], in_=bc_ps[:])
                    nc.vector.tensor_mul(
                        out=yt_sb[:, h, :], in0=yt_ps[:], in1=drecip[:])

                # ---- c_proj for strip s (t tiles 4s..4s+3) ----
                for tr in range(4):
                    tt = 4 * s + tr
                    for os_ in range(4):
                        ps = acc.tile([P, 512], F32, tag="acc")
                        for hh in range(HPC):
                            nc.tensor.matmul(
                                ps[:],
                                lhsT=yt_sb[:, hh, tr * P:(tr + 1) * P],
                                rhs=wc_sb[:, hh, os_ * 512:(os_ + 1) * 512],
                                start=(hh == 0), stop=(hh == HPC - 1))
                        o_sb = op.tile([P, 512], F32, tag="out_sb")
                        nc.vector.tensor_copy(out=o_sb[:], in_=ps[:])
                        nc.sync.dma_start(out_v[:, tt, os_], o_sb[:])

    nc.compile()
    return nc


def make_masks():
    r = np.arange(P)[:, None]
    c = np.arange(512)[None, :]
    return np.ascontiguousarray(
        np.stack([(c >= 128 * b + r) for b in range(4)]).astype(np.float32))


def make_in_maps(x, Wq, Wkv, Wc):
    masks = make_masks()
    in_maps = []
    for core in range(8):
        b, g = core // 4, core % 4
        in_maps.append({
            "xt": np.ascontiguousarray(np.asarray(x[b]).T),
            "wq": np.ascontiguousarray(np.asarray(Wq[:, 512 * g:512 * (g + 1)])),
            "wk": np.ascontiguousarray(np.asarray(Wkv[:, 128 * g:128 * (g + 1)])),
            "wv": np.ascontiguousarray(
                np.asarray(Wkv[:, 512 + 128 * g:512 + 128 * (g + 1)])),
            "wc": np.ascontiguousarray(np.asarray(Wc[512 * g:512 * (g + 1), :])),
            "mask": masks,
            "ones": np.ones((P, P), np.float32),
        })
    return in_maps


_NC_CACHE = {}


def _get_nc():
    if "nc" not in _NC_CACHE:
        _NC_CACHE["nc"] = build_nc()
    return _NC_CACHE["nc"]


def run(x, Wq, Wkv, Wc, trace=False, **kwargs):
    from concourse.bass_utils import run_bass_kernel_spmd
    nc = _get_nc()
    in_maps = make_in_maps(x, Wq, Wkv, Wc)
    res = run_bass_kernel_spmd(nc, in_maps, list(range(8)), trace=trace, **kwargs)
    B, T, C_ = x.shape
    out = np.empty((B, T, C_), np.float32)
    for b in range(B):
        acc = res.results[4 * b]["out"].astype(np.float32)
        for g in range(1, 4):
            acc = acc + res.results[4 * b + g]["out"]
        out[b] = acc
    return out, res


def kernel(x, Wq, Wkv, Wc):
    out, _ = run(x, Wq, Wkv, Wc, trace=False)
    return out
